# revision 1
# baseline (speedup 1.0000x reference)
"""Trainium2 Bass kernel for nn_DZSpecimenClfToy.

Reference computation (per batch item b, B=8, one NeuronCore each):
  1. tv = bilinear_resize(topview[b], (3,64,64) -> (3,4,4))   # fixed 2x2 avg of rows/cols {7,8},{23,24},{39,40},{55,56}
  2. coords = sigmoid(tv.flat @ W1.T + b1).reshape(N,2)       # N=4096
  3. patch top-left tl = coords*2043 (+2-2); all 16 output px of a 4x4
     patch share one bilinear fraction pair -> 5x5 pixel support
  4. out[b] = bilinear_crops.flat @ W2.T + b2                 # [2]

Sharding: data-parallel over batch across 8 cores; weights replicated.

Gather strategy: the toolchain's indirect DMA only supports ONE address per
partition per instruction, so the host uploads the search view in an
overlapped-band layout: 511 bands of 8 rows (stride 4), each stored
[col][row_in_band][ch]. A patch's 5x5x3 support is then one contiguous
111-float run starting at triple index b*16384 + c0*8 + s (b=r0//4,
s=r0%4), and the extraction offsets d*24+(i+di)*3+c are independent of s.
4096 patches = 32 indirect DMAs of [128 partitions x 1 address].
"""
import functools
from contextlib import ExitStack

import numpy as np

import concourse.bass as bass
import concourse.tile as tile
from concourse import bacc, mybir
import concourse.bass_utils as bass_utils
from concourse.bass import IndirectOffsetOnAxis

F32 = mybir.dt.float32
I32 = mybir.dt.int32
ALU = mybir.AluOpType
ACT = mybir.ActivationFunctionType
AX = mybir.AxisListType

B = 8          # batch == number of cores
H = W = 2048   # search view height/width
N = 4096       # patches per item
PS = 4         # patch size
NCLS = 2       # classes
P = 128        # partitions
TPP = N // P   # patches per partition = 32

NBAND = 511            # bands of 8 rows, stride 4: rows 4b..4b+7
BANDTRIP = W * 8       # pixel-triples per band = 16384
SEG = 111              # gathered f32 per patch (covers d*24+(i+di)*3+c <= 110)
SEGP = 128             # SBUF stride per patch segment
MAGIC = 8388608.0      # 2**23


def build_program(num_devices: int, svh: int, svw: int, debug: bool = False):
    pad = float(svh - 1 - PS)  # 2043
    assert svh == H and svw == W, (svh, svw)

    nc = bacc.Bacc("TRN2", target_bir_lowering=False, debug=False,
                   enable_asserts=False, num_devices=num_devices)

    tv = nc.dram_tensor("tv", [3, 64, 64], F32, kind="ExternalInput").ap()
    svb = nc.dram_tensor("svb", [NBAND * BANDTRIP, 3], F32, kind="ExternalInput").ap()
    w1 = nc.dram_tensor("W1", [2 * N, 48], F32, kind="ExternalInput").ap()
    b1 = nc.dram_tensor("b1", [2 * N], F32, kind="ExternalInput").ap()
    w2 = nc.dram_tensor("W2p", [NCLS, N * PS * PS * 3], F32, kind="ExternalInput").ap()
    b2 = nc.dram_tensor("b2", [NCLS], F32, kind="ExternalInput").ap()
    out = nc.dram_tensor("out", [1, NCLS], F32, kind="ExternalOutput").ap()

    dbg = {}
    if debug:
        dbg["s"] = nc.dram_tensor("dbg_s", [P, 2 * TPP], F32, kind="ExternalOutput").ap()
        dbg["idx"] = nc.dram_tensor("dbg_idx", [P, TPP], I32, kind="ExternalOutput").ap()
        dbg["S"] = nc.dram_tensor("dbg_S", [P, TPP * SEGP], F32, kind="ExternalOutput").ap()
        dbg["U"] = nc.dram_tensor("dbg_U", [P, TPP * 48], F32, kind="ExternalOutput").ap()

    with tile.TileContext(nc) as tc:
        with ExitStack() as ctx:
            pool = ctx.enter_context(tc.tile_pool(name="main", bufs=1))

            # ---- input DMAs -------------------------------------------------
            # Topview rows {7,8},{23,24},{39,40},{55,56}: each pair is 128
            # contiguous floats starting at row 7 of each 16-row group.
            A = pool.tile([1, 1536], F32)          # [(c,k), r01*64]
            tv_sel = tv.rearrange("c (k s) w -> c k (s w)", s=16)[:, :, 7 * 64:9 * 64]
            nc.sync.dma_start(A[:].rearrange("p (c k x) -> p c k x", c=3, k=4),
                              tv_sel.unsqueeze(0))

            W1sb = pool.tile([P, 64 * 48], F32)    # row g=p*64+j at [p, j*48:...]
            nc.sync.dma_start(W1sb[:], w1.rearrange("(p j) k -> p (j k)", p=P))

            b1sb = pool.tile([P, 64], F32)
            nc.sync.dma_start(b1sb[:], b1.rearrange("(p j) -> p j", p=P))

            W2sb = pool.tile([P, NCLS * 1536], F32)  # [p, c*1536+x] = W2p[c, p*1536+x]
            nc.sync.dma_start(W2sb[:].rearrange("p (c x) -> p c x", c=NCLS),
                              w2.rearrange("c (p x) -> p c x", p=P))

            b2sb = pool.tile([1, NCLS], F32)
            nc.sync.dma_start(b2sb[:], b2.unsqueeze(0))

            # ---- topview 64x64 -> 4x4 resize, flatten, scale ---------------
            V = pool.tile([1, 768], F32)           # [(c,k), 64] row-pair sums
            A4 = A[:].rearrange("p (ck r w) -> p ck r w", ck=12, r=2)
            nc.vector.tensor_add(V[:].rearrange("p (ck w) -> p ck w", ck=12),
                                 A4[:, :, 0, :], A4[:, :, 1, :])
            F48 = pool.tile([1, 48], F32)
            V4 = V[:].rearrange("p (ck g s) -> p ck g s", ck=12, g=4)
            nc.vector.tensor_add(F48[:].rearrange("p (ck g) -> p ck g", ck=12),
                                 V4[:, :, :, 7], V4[:, :, :, 8])
            flatF = pool.tile([1, 48], F32)
            nc.vector.tensor_scalar_mul(flatF[:], F48[:], 0.25)

            # broadcast flat to all partitions (bounce through DRAM)
            dram_pool = ctx.enter_context(tc.tile_pool(name="dram", bufs=1, space="DRAM"))
            fdram = dram_pool.tile([1, 48], F32)
            nc.sync.dma_start(fdram[:], flatF[:])
            flatb = pool.tile([P, 48], F32)
            nc.sync.dma_start(flatb[:], fdram[:].to_broadcast((P, 48)))

            # ---- coords = sigmoid(W1 @ flat + b1), [128, 64] ---------------
            mul1 = pool.tile([P, 64 * 48], F32)
            nc.vector.tensor_mul(mul1[:].rearrange("p (j k) -> p j k", j=64),
                                 W1sb[:].rearrange("p (j k) -> p j k", j=64),
                                 flatb[:].unsqueeze(1).to_broadcast((P, 64, 48)))
            pre = pool.tile([P, 64], F32)
            nc.vector.reduce_sum(pre[:].unsqueeze(2),
                                 mul1[:].rearrange("p (j k) -> p j k", j=64),
                                 axis=AX.X)
            preb = pool.tile([P, 64], F32)
            nc.vector.tensor_add(preb[:], pre[:], b1sb[:])
            sg = pool.tile([P, 64], F32)
            nc.scalar.activation(sg[:], preb[:], ACT.Sigmoid)
            if debug:
                nc.sync.dma_start(dbg["s"], sg[:])

            # ---- patch top-left corners and fractions ----------------------
            s3 = sg[:].rearrange("p (t two) -> p t two", two=2)

            def floor_to(dst, src, tag):
                """dst = floor(src), src >= 0, via round-to-nearest + correction."""
                rnd = pool.tile([P, TPP], F32, tag=f"rnd{tag}")
                nc.vector.tensor_scalar(rnd[:], src, MAGIC, MAGIC,
                                        op0=ALU.add, op1=ALU.subtract)
                gt = pool.tile([P, TPP], F32, tag=f"gt{tag}")
                nc.vector.tensor_tensor(gt[:], rnd[:], src, op=ALU.is_gt)
                nc.vector.tensor_sub(dst, rnd[:], gt[:])

            def corner(sel):
                xs = pool.tile([P, TPP], F32, tag=f"xs{sel}")
                nc.vector.tensor_scalar(xs[:], s3[:, :, sel], pad, float(PS // 2),
                                        op0=ALU.mult, op1=ALU.add)
                tl = pool.tile([P, TPP], F32, tag=f"tl{sel}")
                nc.vector.tensor_scalar_sub(tl[:], xs[:], float(PS // 2))
                c0 = pool.tile([P, TPP], F32, tag=f"c0{sel}")
                floor_to(c0[:], tl[:], f"c{sel}")
                fr = pool.tile([P, TPP], F32, tag=f"fr{sel}")
                nc.vector.tensor_sub(fr[:], tl[:], c0[:])
                return c0, fr

            r0f, fr = corner(0)   # rows
            c0f, fc = corner(1)   # cols

            # ---- gather index (pixel-triple units, +MAGIC bias) ------------
            # band b = r0//4, s = r0%4, idx = b*16384 + c0*8 + s
            bq = pool.tile([P, TPP], F32)
            nc.vector.tensor_scalar_mul(bq[:], r0f[:], 0.25)
            bf = pool.tile([P, TPP], F32)
            floor_to(bf[:], bq[:], "b")
            sres = pool.tile([P, TPP], F32)        # s = r0 - 4b
            nc.vector.tensor_scalar(sres[:], bf[:], -4.0, None, op0=ALU.mult)
            nc.vector.tensor_add(sres[:], sres[:], r0f[:])
            t1 = pool.tile([P, TPP], F32)
            nc.vector.tensor_scalar(t1[:], bf[:], float(BANDTRIP), MAGIC,
                                    op0=ALU.mult, op1=ALU.add)
            t2 = pool.tile([P, TPP], F32)
            nc.vector.tensor_scalar(t2[:], c0f[:], 8.0, None, op0=ALU.mult)
            nc.vector.tensor_add(t2[:], t2[:], sres[:])
            idxf = pool.tile([P, TPP], F32)
            nc.vector.tensor_add(idxf[:], t1[:], t2[:])
            idxi = pool.tile([P, TPP], I32)
            nc.vector.tensor_single_scalar(idxi[:], idxf[:].bitcast(I32),
                                           0x007FFFFF, op=ALU.bitwise_and)
            if debug:
                nc.sync.dma_start(dbg["idx"], idxi[:])

            # ---- gather: one 111-float run per patch, 32 x [128 x 1] -------
            S = pool.tile([P, TPP * SEGP], F32)
            if debug:
                nc.vector.memset(S[:], 0.0)  # the dbg_S dump reads the padding
            for t in range(TPP):
                nc.gpsimd.indirect_dma_start(
                    out=S[:, t * SEGP: t * SEGP + SEG],
                    out_offset=None,
                    in_=svb,
                    in_offset=IndirectOffsetOnAxis(ap=idxi[:, t:t + 1], axis=0),
                )
            if debug:
                nc.sync.dma_start(dbg["S"], S[:])

            # ---- bilinear combine ------------------------------------------
            # segment layout per patch: elem(d, m, c) at d*24 + m*3 + c,
            # m = i + di (0..4). Row interp over di, col interp over dj.
            Sv = S[:].rearrange("p (t x) -> p t x", t=TPP)

            def seg_view(off):
                # [p, t, d(5 cols, stride 24), 12 = (i,c)] at element offset off
                return Sv[:, :, off:off + 120].rearrange(
                    "p t (d e) -> p t d e", d=5)[:, :, :, 0:12]

            D1 = pool.tile([P, TPP * 60], F32)
            D1v = D1[:].rearrange("p (t d e) -> p t d e", t=TPP, d=5)
            nc.vector.tensor_sub(D1v, seg_view(3), seg_view(0))
            M1 = pool.tile([P, TPP * 60], F32)
            M1v = M1[:].rearrange("p (t d e) -> p t d e", t=TPP, d=5)
            nc.vector.tensor_mul(M1v, D1v,
                                 fr[:].unsqueeze(2).unsqueeze(3).to_broadcast((P, TPP, 5, 12)))
            T = pool.tile([P, TPP * 60], F32)
            nc.vector.tensor_add(T[:].rearrange("p (t d e) -> p t d e", t=TPP, d=5),
                                 M1v, seg_view(0))

            # col interp: U[t, j, i, c] = T(d=j) + fc*(T(d=j+1) - T(d=j))
            Tv = T[:].rearrange("p (t x) -> p t x", t=TPP)
            T0 = Tv[:, :, 0:48].rearrange("p t (d e) -> p t d e", d=4)
            T12 = Tv[:, :, 12:60].rearrange("p t (d e) -> p t d e", d=4)
            D2 = pool.tile([P, TPP * 48], F32)
            D2v = D2[:].rearrange("p (t d e) -> p t d e", t=TPP, d=4)
            nc.vector.tensor_sub(D2v, T12, T0)
            M2 = pool.tile([P, TPP * 48], F32)
            M2v = M2[:].rearrange("p (t d e) -> p t d e", t=TPP, d=4)
            nc.vector.tensor_mul(M2v, D2v,
                                 fc[:].unsqueeze(2).unsqueeze(3).to_broadcast((P, TPP, 4, 12)))
            U = pool.tile([P, TPP * 48], F32)
            nc.vector.tensor_add(U[:].rearrange("p (t d e) -> p t d e", t=TPP, d=4),
                                 M2v, T0)
            if debug:
                nc.sync.dma_start(dbg["U"], U[:])

            # ---- classifier: out[c] = sum(U * W2p[c]) + b2 -----------------
            mW2 = pool.tile([P, NCLS * 1536], F32)
            nc.vector.tensor_mul(mW2[:].rearrange("p (c x) -> p c x", c=NCLS),
                                 W2sb[:].rearrange("p (c x) -> p c x", c=NCLS),
                                 U[:].unsqueeze(1).to_broadcast((P, NCLS, 1536)))
            r2 = pool.tile([P, NCLS], F32)
            nc.vector.reduce_sum(r2[:].unsqueeze(2),
                                 mW2[:].rearrange("p (c x) -> p c x", c=NCLS),
                                 axis=AX.X)
            ppool = ctx.enter_context(tc.tile_pool(name="ps", bufs=1, space="PSUM"))
            ones = pool.tile([P, 1], F32)
            nc.vector.memset(ones[:], 1.0)
            osum = ppool.tile([1, NCLS], F32)
            nc.tensor.matmul(out=osum[:], lhsT=ones[:], rhs=r2[:], start=True, stop=True)
            ofin = pool.tile([1, NCLS], F32)
            nc.vector.tensor_add(ofin[:], osum[:], b2sb[:])
            nc.sync.dma_start(out, ofin[:])

    nc.compile()
    return nc


@functools.lru_cache(maxsize=2)
def _compiled(num_devices: int, svh: int, svw: int, debug: bool = False):
    return build_program(num_devices, svh, svw, debug)


def band_layout(img: np.ndarray) -> np.ndarray:
    """[2048, 2048, 3] -> [511*16384, 3]: 8-row bands at stride 4, [col][row][ch]."""
    sw = np.lib.stride_tricks.sliding_window_view(img, 8, axis=0)  # [2041, 2048, 3, 8]
    sb = sw[::4]                                                   # [511, 2048, 3, 8]
    return np.ascontiguousarray(sb.transpose(0, 1, 3, 2)).reshape(-1, 3)


def permute_w2(W2: np.ndarray) -> np.ndarray:
    """Reorder per-patch (i, j, c) -> (j, i, c) to match the kernel's U layout."""
    return np.ascontiguousarray(
        W2.reshape(NCLS, N, PS, PS, 3).transpose(0, 1, 3, 2, 4)).reshape(NCLS, -1)


def make_in_maps(topview, search_views, W1, b1, W2, b2):
    W1 = np.ascontiguousarray(W1, np.float32)
    b1 = np.ascontiguousarray(b1, np.float32)
    W2p = permute_w2(np.ascontiguousarray(W2, np.float32))
    b2 = np.ascontiguousarray(b2, np.float32)
    return [{
        "tv": np.ascontiguousarray(topview[i], np.float32),
        "svb": band_layout(np.ascontiguousarray(search_views[i], np.float32)),
        "W1": W1, "b1": b1, "W2p": W2p, "b2": b2,
    } for i in range(topview.shape[0])]


def kernel(topview, search_views, W1, b1, W2, b2, svh, svw):
    svh, svw = int(svh), int(svw)
    nc = _compiled(B, svh, svw)
    in_maps = make_in_maps(topview, search_views, W1, b1, W2, b2)
    res = bass_utils.run_bass_kernel_spmd(nc, in_maps, core_ids=list(range(B)))
    return np.concatenate([res.results[i]["out"] for i in range(B)], axis=0)



# revision 3
# speedup vs baseline: 1.5293x; 1.5293x over previous
"""Trainium2 Bass kernel for nn_DZSpecimenClfToy.

Reference computation (per batch item b, B=8, one NeuronCore each):
  1. tv = bilinear_resize(topview[b], (3,64,64) -> (3,4,4))   # fixed 2x2 avg of rows/cols {7,8},{23,24},{39,40},{55,56}
  2. coords = sigmoid(tv.flat @ W1.T + b1).reshape(N,2)       # N=4096
  3. patch top-left tl = coords*2043 (+2-2); all 16 output px of a 4x4
     patch share one bilinear fraction pair -> 5x5 pixel support
  4. out[b] = bilinear_crops.flat @ W2.T + b2                 # [2]

Sharding: data-parallel over batch across 8 cores; weights replicated.

Gather strategy: the toolchain's indirect DMA only supports ONE address per
partition per instruction, so the host uploads the search view in an
overlapped-band layout: 511 bands of 8 rows (stride 4), each stored
[col][row_in_band][ch]. A patch's 5x5x3 support is then one contiguous
111-float run starting at triple index b*16384 + c0*8 + s (b=r0//4,
s=r0%4), and the extraction offsets d*24+(i+di)*3+c are independent of s.
4096 patches = 32 indirect DMAs of [128 partitions x 1 address].
"""
import functools
from contextlib import ExitStack

import numpy as np

import concourse.bass as bass
import concourse.tile as tile
from concourse import bacc, mybir
import concourse.bass_utils as bass_utils
from concourse.bass import IndirectOffsetOnAxis

F32 = mybir.dt.float32
I32 = mybir.dt.int32
ALU = mybir.AluOpType
ACT = mybir.ActivationFunctionType
AX = mybir.AxisListType

B = 8          # batch == number of cores
H = W = 2048   # search view height/width
N = 4096       # patches per item
PS = 4         # patch size
NCLS = 2       # classes
P = 128        # partitions
TPP = N // P   # patches per partition = 32

NBAND = 511            # bands of 8 rows, stride 4: rows 4b..4b+7
BANDTRIP = W * 8       # pixel-triples per band = 16384
SEG = 111              # gathered f32 per patch (covers d*24+(i+di)*3+c <= 110)
SEGP = 128             # SBUF stride per patch segment
MAGIC = 8388608.0      # 2**23


def build_program(num_devices: int, svh: int, svw: int, debug: bool = False):
    pad = float(svh - 1 - PS)  # 2043
    assert svh == H and svw == W, (svh, svw)

    nc = bacc.Bacc("TRN2", target_bir_lowering=False, debug=False,
                   enable_asserts=False, num_devices=num_devices)

    tv = nc.dram_tensor("tv", [3, 64, 64], F32, kind="ExternalInput").ap()
    svb = nc.dram_tensor("svb", [NBAND * BANDTRIP, 3], F32, kind="ExternalInput").ap()
    w1 = nc.dram_tensor("W1", [2 * N, 48], F32, kind="ExternalInput").ap()
    b1 = nc.dram_tensor("b1", [2 * N], F32, kind="ExternalInput").ap()
    w2 = nc.dram_tensor("W2p", [NCLS, N * PS * PS * 3], F32, kind="ExternalInput").ap()
    b2 = nc.dram_tensor("b2", [NCLS], F32, kind="ExternalInput").ap()
    out = nc.dram_tensor("out", [1, NCLS], F32, kind="ExternalOutput").ap()

    dbg = {}
    if debug:
        dbg["s"] = nc.dram_tensor("dbg_s", [P, 2 * TPP], F32, kind="ExternalOutput").ap()
        dbg["idx"] = nc.dram_tensor("dbg_idx", [P, TPP], I32, kind="ExternalOutput").ap()
        dbg["S"] = nc.dram_tensor("dbg_S", [P, TPP * SEGP], F32, kind="ExternalOutput").ap()
        dbg["U"] = nc.dram_tensor("dbg_U", [P, TPP * 48], F32, kind="ExternalOutput").ap()

    with tile.TileContext(nc) as tc:
        with ExitStack() as ctx:
            pool = ctx.enter_context(tc.tile_pool(name="main", bufs=1))

            # ---- input DMAs -------------------------------------------------
            # Topview rows {7,8},{23,24},{39,40},{55,56}: each pair is 128
            # contiguous floats starting at row 7 of each 16-row group.
            A = pool.tile([1, 1536], F32)          # [(c,k), r01*64]
            tv_sel = tv.rearrange("c (k s) w -> c k (s w)", s=16)[:, :, 7 * 64:9 * 64]
            nc.sync.dma_start(A[:].rearrange("p (c k x) -> p c k x", c=3, k=4),
                              tv_sel.unsqueeze(0))

            W1sb = pool.tile([P, 64 * 48], F32)    # row g=p*64+j at [p, j*48:...]
            nc.sync.dma_start(W1sb[:], w1.rearrange("(p j) k -> p (j k)", p=P))

            b1sb = pool.tile([P, 64], F32)
            nc.sync.dma_start(b1sb[:], b1.rearrange("(p j) -> p j", p=P))

            W2sb = pool.tile([P, NCLS * 1536], F32)  # [p, c*1536+x] = W2p[c, p*1536+x]
            nc.sync.dma_start(W2sb[:].rearrange("p (c x) -> p c x", c=NCLS),
                              w2.rearrange("c (p x) -> p c x", p=P))

            b2sb = pool.tile([1, NCLS], F32)
            nc.sync.dma_start(b2sb[:], b2.unsqueeze(0))

            # ---- topview 64x64 -> 4x4 resize, flatten, scale ---------------
            V = pool.tile([1, 768], F32)           # [(c,k), 64] row-pair sums
            A4 = A[:].rearrange("p (ck r w) -> p ck r w", ck=12, r=2)
            nc.vector.tensor_add(V[:].rearrange("p (ck w) -> p ck w", ck=12),
                                 A4[:, :, 0, :], A4[:, :, 1, :])
            F48 = pool.tile([1, 48], F32)
            V4 = V[:].rearrange("p (ck g s) -> p ck g s", ck=12, g=4)
            nc.vector.tensor_add(F48[:].rearrange("p (ck g) -> p ck g", ck=12),
                                 V4[:, :, :, 7], V4[:, :, :, 8])
            flatF = pool.tile([1, 48], F32)
            nc.vector.tensor_scalar_mul(flatF[:], F48[:], 0.25)

            # broadcast flat to all partitions (bounce through DRAM)
            dram_pool = ctx.enter_context(tc.tile_pool(name="dram", bufs=1, space="DRAM"))
            fdram = dram_pool.tile([1, 48], F32)
            nc.sync.dma_start(fdram[:], flatF[:])
            flatb = pool.tile([P, 48], F32)
            nc.sync.dma_start(flatb[:], fdram[:].to_broadcast((P, 48)))

            # ---- coords = sigmoid(W1 @ flat + b1), [128, 64] ---------------
            mul1 = pool.tile([P, 64 * 48], F32)
            nc.vector.tensor_mul(mul1[:].rearrange("p (j k) -> p j k", j=64),
                                 W1sb[:].rearrange("p (j k) -> p j k", j=64),
                                 flatb[:].unsqueeze(1).to_broadcast((P, 64, 48)))
            pre = pool.tile([P, 64], F32)
            nc.vector.reduce_sum(pre[:].unsqueeze(2),
                                 mul1[:].rearrange("p (j k) -> p j k", j=64),
                                 axis=AX.X)
            preb = pool.tile([P, 64], F32)
            nc.vector.tensor_add(preb[:], pre[:], b1sb[:])
            sg = pool.tile([P, 64], F32)
            nc.scalar.activation(sg[:], preb[:], ACT.Sigmoid)
            if debug:
                nc.sync.dma_start(dbg["s"], sg[:])

            # ---- patch top-left corners and fractions ----------------------
            s3 = sg[:].rearrange("p (t two) -> p t two", two=2)

            def floor_to(dst, src, tag):
                """dst = floor(src), src >= 0, via round-to-nearest + correction."""
                rnd = pool.tile([P, TPP], F32, tag=f"rnd{tag}")
                nc.vector.tensor_scalar(rnd[:], src, MAGIC, MAGIC,
                                        op0=ALU.add, op1=ALU.subtract)
                gt = pool.tile([P, TPP], F32, tag=f"gt{tag}")
                nc.vector.tensor_tensor(gt[:], rnd[:], src, op=ALU.is_gt)
                nc.vector.tensor_sub(dst, rnd[:], gt[:])

            def corner(sel):
                xs = pool.tile([P, TPP], F32, tag=f"xs{sel}")
                nc.vector.tensor_scalar(xs[:], s3[:, :, sel], pad, float(PS // 2),
                                        op0=ALU.mult, op1=ALU.add)
                tl = pool.tile([P, TPP], F32, tag=f"tl{sel}")
                nc.vector.tensor_scalar_sub(tl[:], xs[:], float(PS // 2))
                c0 = pool.tile([P, TPP], F32, tag=f"c0{sel}")
                floor_to(c0[:], tl[:], f"c{sel}")
                fr = pool.tile([P, TPP], F32, tag=f"fr{sel}")
                nc.vector.tensor_sub(fr[:], tl[:], c0[:])
                return c0, fr

            r0f, fr = corner(0)   # rows
            c0f, fc = corner(1)   # cols

            # ---- gather index (pixel-triple units, +MAGIC bias) ------------
            # band b = r0//4, s = r0%4, idx = b*16384 + c0*8 + s
            bq = pool.tile([P, TPP], F32)
            nc.vector.tensor_scalar_mul(bq[:], r0f[:], 0.25)
            bf = pool.tile([P, TPP], F32)
            floor_to(bf[:], bq[:], "b")
            sres = pool.tile([P, TPP], F32)        # s = r0 - 4b
            nc.vector.tensor_scalar(sres[:], bf[:], -4.0, None, op0=ALU.mult)
            nc.vector.tensor_add(sres[:], sres[:], r0f[:])
            t1 = pool.tile([P, TPP], F32)
            nc.vector.tensor_scalar(t1[:], bf[:], float(BANDTRIP), MAGIC,
                                    op0=ALU.mult, op1=ALU.add)
            t2 = pool.tile([P, TPP], F32)
            nc.vector.tensor_scalar(t2[:], c0f[:], 8.0, None, op0=ALU.mult)
            nc.vector.tensor_add(t2[:], t2[:], sres[:])
            idxf = pool.tile([P, TPP], F32)
            nc.vector.tensor_add(idxf[:], t1[:], t2[:])
            idxi = pool.tile([P, TPP], I32)
            nc.vector.tensor_single_scalar(idxi[:], idxf[:].bitcast(I32),
                                           0x007FFFFF, op=ALU.bitwise_and)
            if debug:
                nc.sync.dma_start(dbg["idx"], idxi[:])

            # ---- gather: one 111-float run per patch, 32 x [128 x 1] -------
            S = pool.tile([P, TPP * SEGP], F32)
            if debug:
                nc.vector.memset(S[:], 0.0)  # the dbg_S dump reads the padding
            for t in range(TPP):
                nc.gpsimd.indirect_dma_start(
                    out=S[:, t * SEGP: t * SEGP + SEG],
                    out_offset=None,
                    in_=svb,
                    in_offset=IndirectOffsetOnAxis(ap=idxi[:, t:t + 1], axis=0),
                )
            if debug:
                nc.sync.dma_start(dbg["S"], S[:])

            # ---- bilinear combine ------------------------------------------
            # segment layout per patch: elem(d, m, c) at d*24 + m*3 + c,
            # m = i + di (0..4). Row interp over di, col interp over dj.
            Sv = S[:].rearrange("p (t x) -> p t x", t=TPP)

            def seg_view(off):
                # [p, t, d(5 cols, stride 24), 12 = (i,c)] at element offset off
                return Sv[:, :, off:off + 120].rearrange(
                    "p t (d e) -> p t d e", d=5)[:, :, :, 0:12]

            D1 = pool.tile([P, TPP * 60], F32)
            D1v = D1[:].rearrange("p (t d e) -> p t d e", t=TPP, d=5)
            nc.vector.tensor_sub(D1v, seg_view(3), seg_view(0))
            M1 = pool.tile([P, TPP * 60], F32)
            M1v = M1[:].rearrange("p (t d e) -> p t d e", t=TPP, d=5)
            nc.vector.tensor_mul(M1v, D1v,
                                 fr[:].unsqueeze(2).unsqueeze(3).to_broadcast((P, TPP, 5, 12)))
            T = pool.tile([P, TPP * 60], F32)
            nc.vector.tensor_add(T[:].rearrange("p (t d e) -> p t d e", t=TPP, d=5),
                                 M1v, seg_view(0))

            # col interp: U[t, j, i, c] = T(d=j) + fc*(T(d=j+1) - T(d=j))
            Tv = T[:].rearrange("p (t x) -> p t x", t=TPP)
            T0 = Tv[:, :, 0:48].rearrange("p t (d e) -> p t d e", d=4)
            T12 = Tv[:, :, 12:60].rearrange("p t (d e) -> p t d e", d=4)
            D2 = pool.tile([P, TPP * 48], F32)
            D2v = D2[:].rearrange("p (t d e) -> p t d e", t=TPP, d=4)
            nc.vector.tensor_sub(D2v, T12, T0)
            M2 = pool.tile([P, TPP * 48], F32)
            M2v = M2[:].rearrange("p (t d e) -> p t d e", t=TPP, d=4)
            nc.vector.tensor_mul(M2v, D2v,
                                 fc[:].unsqueeze(2).unsqueeze(3).to_broadcast((P, TPP, 4, 12)))
            U = pool.tile([P, TPP * 48], F32)
            nc.vector.tensor_add(U[:].rearrange("p (t d e) -> p t d e", t=TPP, d=4),
                                 M2v, T0)
            if debug:
                nc.sync.dma_start(dbg["U"], U[:])

            # ---- classifier: out[c] = sum(U * W2p[c]) + b2 -----------------
            mW2 = pool.tile([P, NCLS * 1536], F32)
            nc.vector.tensor_mul(mW2[:].rearrange("p (c x) -> p c x", c=NCLS),
                                 W2sb[:].rearrange("p (c x) -> p c x", c=NCLS),
                                 U[:].unsqueeze(1).to_broadcast((P, NCLS, 1536)))
            r2 = pool.tile([P, NCLS], F32)
            nc.vector.reduce_sum(r2[:].unsqueeze(2),
                                 mW2[:].rearrange("p (c x) -> p c x", c=NCLS),
                                 axis=AX.X)
            ppool = ctx.enter_context(tc.tile_pool(name="ps", bufs=1, space="PSUM"))
            ones = pool.tile([P, 1], F32)
            nc.vector.memset(ones[:], 1.0)
            osum = ppool.tile([1, NCLS], F32)
            nc.tensor.matmul(out=osum[:], lhsT=ones[:], rhs=r2[:], start=True, stop=True)
            ofin = pool.tile([1, NCLS], F32)
            nc.vector.tensor_add(ofin[:], osum[:], b2sb[:])
            nc.sync.dma_start(out, ofin[:])

    nc.compile()
    return nc


@functools.lru_cache(maxsize=2)
def _compiled(num_devices: int, svh: int, svw: int, debug: bool = False):
    return build_program(num_devices, svh, svw, debug)


def band_layout(img: np.ndarray) -> np.ndarray:
    """[2048, 2048, 3] -> [511*16384, 3]: 8-row bands at stride 4, [col][row][ch]."""
    sw = np.lib.stride_tricks.sliding_window_view(img, 8, axis=0)  # [2041, 2048, 3, 8]
    sb = sw[::4]                                                   # [511, 2048, 3, 8]
    return np.ascontiguousarray(sb.transpose(0, 1, 3, 2)).reshape(-1, 3)


def permute_w2(W2: np.ndarray) -> np.ndarray:
    """Reorder per-patch (i, j, c) -> (j, i, c) to match the kernel's U layout."""
    return np.ascontiguousarray(
        W2.reshape(NCLS, N, PS, PS, 3).transpose(0, 1, 3, 2, 4)).reshape(NCLS, -1)


def make_in_maps(topview, search_views, W1, b1, W2, b2):
    W1 = np.ascontiguousarray(W1, np.float32)
    b1 = np.ascontiguousarray(b1, np.float32)
    W2p = permute_w2(np.ascontiguousarray(W2, np.float32))
    b2 = np.ascontiguousarray(b2, np.float32)
    return [{
        "tv": np.ascontiguousarray(topview[i], np.float32),
        "svb": band_layout(np.ascontiguousarray(search_views[i], np.float32)),
        "W1": W1, "b1": b1, "W2p": W2p, "b2": b2,
    } for i in range(topview.shape[0])]


def kernel(topview, search_views, W1, b1, W2, b2, svh, svw):
    svh, svw = int(svh), int(svw)
    nc = _compiled(B, svh, svw)
    in_maps = make_in_maps(topview, search_views, W1, b1, W2, b2)
    res = bass_utils.run_bass_kernel_spmd(nc, in_maps, core_ids=list(range(B)))
    return np.concatenate([res.results[i]["out"] for i in range(B)], axis=0)



# revision 4
# speedup vs baseline: 1.6386x; 1.0715x over previous
"""Trainium2 Bass kernel for nn_DZSpecimenClfToy — v2.

Per batch item b (B=8, one NeuronCore each):
  1. tv = bilinear_resize(topview[b], (3,64,64)->(3,4,4)) = 0.25*(2x2 sums of
     rows/cols {7,8} of each 16-group); flat[48] in (c,h,w) order.
  2. pre[g] = W1[g,:]@flat + b1[g] (g=8192) on the PE: 32 matmuls of
     lhsT[98,128] x rhs[98,2] (k=0..47 -> W1 row (p*64+2q), k=48..95 ->
     row (p*64+2q+1), k=96/97 -> b1 via eye2 columns in rhs).
  3. coords = sigmoid(pre) (ACT, from PSUM); tl = coords*2043; floor/frac via
     the 2^23 magic trick; gather index = r0*2048 + c0.
  4. Search view pre-laid out on host as 5-row bands (fp16):
     svb16[r0*2048 + c0] = img[r0:r0+5, c0, :] -> one patch support =
     5 consecutive 15-elem column blocks = 75 contiguous fp16.
     32 indirect DMAs (one address per partition each) gather all 4096.
  5. Bilinear combine in fp16 on DVE (row lerp then col lerp).
  6. Per-class dot via tensor_tensor_reduce with cross-chunk accumulator
     chaining (init = b2/128); final partition sum via ones-matmul on PE.

Sharding: data-parallel over batch across 8 cores; weights replicated.
"""
import functools
from contextlib import ExitStack

import numpy as np

import concourse.bass as bass
import concourse.tile as tile
from concourse import bacc, mybir
import concourse.bass_utils as bass_utils
from concourse.bass import IndirectOffsetOnAxis

F32 = mybir.dt.float32
F16 = mybir.dt.float16
I32 = mybir.dt.int32
ALU = mybir.AluOpType
ACT = mybir.ActivationFunctionType
AX = mybir.AxisListType

B = 8          # batch == number of cores
H = W = 2048   # search view height/width
N = 4096       # patches per item
PS = 4         # patch size
NCLS = 2       # classes
P = 128        # partitions
TPP = N // P   # patches per partition = 32
NCH = 8        # pipeline chunks
TPC = TPP // NCH  # patches per partition per chunk = 4

NB = H - PS    # 2044 bands of 5 rows (r0 in 0..2043; used r0 <= 2042)
UNIT = 15      # fp16 elems per (band,col) unit: 5 rows x 3 ch
SEG = 75       # gathered fp16 per patch (5 column units)
SEGP = 80      # SBUF stride per patch segment
MAGIC = 8388608.0  # 2**23
PAD = 2043.0   # (svh-1) - ps


def build_program(num_devices: int, svh: int, svw: int):
    assert svh == H and svw == W, (svh, svw)

    nc = bacc.Bacc("TRN2", target_bir_lowering=False, debug=False,
                   enable_asserts=False, num_devices=num_devices)

    tv = nc.dram_tensor("tv", [3, 64, 64], F32, kind="ExternalInput").ap()
    svb = nc.dram_tensor("svb", [NB * W, UNIT], F16, kind="ExternalInput").ap()
    w1pe = nc.dram_tensor("W1pe", [P, TPP * P], F32, kind="ExternalInput").ap()
    w2 = nc.dram_tensor("W2p", [NCLS, N * PS * PS * 3], F16, kind="ExternalInput").ap()
    b2 = nc.dram_tensor("b2", [NCLS], F32, kind="ExternalInput").ap()
    out = nc.dram_tensor("out", [1, NCLS], F32, kind="ExternalOutput").ap()

    with tile.TileContext(nc) as tc:
        with ExitStack() as ctx:
            pool = ctx.enter_context(tc.tile_pool(name="main", bufs=1))
            ppool = ctx.enter_context(tc.tile_pool(name="ps", bufs=1, space="PSUM"))

            # ---- input DMAs ------------------------------------------------
            # Topview rows {7,8},{23,24},{39,40},{55,56}: 128 contiguous floats
            # per (c, rowgroup) starting at row 7 of each 16-row group.
            A = pool.tile([1, 1536], F32)
            tv_sel = tv.rearrange("c (k s) w -> c k (s w)", s=16)[:, :, 7 * 64:9 * 64]
            nc.sync.dma_start(A[:].rearrange("p (c k x) -> p c k x", c=3, k=4),
                              tv_sel.unsqueeze(0))

            W1sb = pool.tile([P, TPP * P], F32)
            nc.sync.dma_start(W1sb[:, 0:2048], w1pe[:, 0:2048])
            nc.sync.dma_start(W1sb[:, 2048:4096], w1pe[:, 2048:4096])

            W2sb = pool.tile([P, NCLS * 1536], F16)  # [p, c*1536 + t*48 + x]
            nc.sync.dma_start(W2sb[:].rearrange("p (c x) -> p c x", c=NCLS),
                              w2.rearrange("c (p x) -> p c x", p=P))

            b2sb = pool.tile([1, NCLS], F32)
            nc.sync.dma_start(b2sb[:], b2.unsqueeze(0))

            ones = pool.tile([P, 1], F32)
            nc.vector.memset(ones[:], 1.0)

            # ---- topview 64x64 -> 4x4 resize -> flat16 ---------------------
            V = pool.tile([1, 768], F32)           # row-pair sums
            A4 = A[:].rearrange("p (ck r w) -> p ck r w", ck=12, r=2)
            nc.vector.tensor_add(V[:].rearrange("p (ck w) -> p ck w", ck=12),
                                 A4[:, :, 0, :], A4[:, :, 1, :])
            F48 = pool.tile([1, 48], F32)
            V4 = V[:].rearrange("p (ck g s) -> p ck g s", ck=12, g=4)
            nc.vector.tensor_add(F48[:].rearrange("p (ck g) -> p ck g", ck=12),
                                 V4[:, :, :, 7], V4[:, :, :, 8])
            # flat2x = [flat*0.25, 1, pad0..; flat*0.25, 1, pad0..] on one
            # partition; rows 49:64 / 113:128 multiply zero lhsT rows.
            flat2x = pool.tile([1, P], F32)
            nc.vector.memset(flat2x[:], 0.0)
            nc.vector.tensor_scalar_mul(flat2x[:, 0:48], F48[:], 0.25)
            nc.vector.memset(flat2x[:, 48:49], 1.0)
            nc.vector.tensor_scalar_mul(flat2x[:, 64:112], F48[:], 0.25)
            nc.vector.memset(flat2x[:, 112:113], 1.0)

            # ---- rhs [128, 2] built on-chip: K=1 matmul transposes flat2x
            # onto partitions, then two 64-aligned DVE column copies.
            # (PSUM banks are scarce: borrow chunk 7's matvec bank — its
            # matmuls overwrite it only after the rhs copies have read it.)
            QPC = TPP // NCH   # matmuls per chunk = 4
            psum_t = [ppool.tile([P, 2 * QPC], F32, name=f"psum{c}")
                      for c in range(NCH)]
            psumT = psum_t[NCH - 1][:, 0:1]
            nc.tensor.matmul(out=psumT, lhsT=flat2x[:], rhs=ones[0:1, 0:1],
                             start=True, stop=True)
            rhs = pool.tile([P, 2], F32)
            nc.vector.memset(rhs[:], 0.0)
            nc.vector.tensor_copy(rhs[0:64, 0:1], psumT[0:64, 0:1])
            nc.vector.tensor_copy(rhs[64:128, 1:2], psumT[64:128, 0:1])

            # ---- PE matvec: psum_c[p, j] = pre[p*64+j] + b1 (per chunk) ----
            for q in range(TPP):
                c, qq = q // QPC, q % QPC
                nc.tensor.matmul(out=psum_t[c][:, 2 * qq:2 * qq + 2],
                                 lhsT=W1sb[:, P * q:P * (q + 1)],
                                 rhs=rhs[:], start=True, stop=True)

            sg = pool.tile([P, 64], F32)
            frh = pool.tile([P, 64], F16)   # fp16 fractions, (t, two) interleaved
            idxi = pool.tile([P, TPP], I32)
            S = pool.tile([P, TPP * SEGP], F16)

            # scratch per chunk
            def cpool(name, w, dt=F32):
                return [pool.tile([P, w], dt, name=f"{name}{c}")
                        for c in range(NCH)]

            tl_t = cpool("tl", 2 * TPC)
            rnd_t = cpool("rnd", 2 * TPC)
            gt_t = cpool("gt", 2 * TPC)
            flr_t = cpool("flr", 2 * TPC)
            t1_t = cpool("t1", TPC)
            ixf_t = cpool("ixf", TPC)

            # combine splits: six halves of 4 patches + four of 2 (smaller
            # tail chunks shrink the exposed post-train combine time)
            HSIZES = [4, 4, 4, 4, 4, 4, 2, 2, 2, 2]
            HSTART = [sum(HSIZES[:i]) for i in range(len(HSIZES))]
            NH = len(HSIZES)

            def hpool(name, scale, dt=F16):
                return [pool.tile([P, HSIZES[h] * scale], dt, name=f"{name}{h}")
                        for h in range(NH)]

            D1_t = hpool("D1", 60)
            M1_t = hpool("M1", 60)
            T_t = hpool("T", 60)
            D2_t = hpool("D2", 48)
            M2_t = hpool("M2", 48)
            U_t = hpool("U", 48)
            dummy = pool.tile([P, NCLS * 4 * 48], F16)
            r2_t = [pool.tile([P, NCLS], F32, name=f"r2{h}") for h in range(NH)]
            acc_t = [pool.tile([P, NCLS], F32, name=f"acc{h}") for h in range(NH)]

            # ---- per-chunk: sigmoid -> coords -> gather indices ------------
            for c in range(NCH):
                js = slice(2 * TPC * c, 2 * TPC * (c + 1))
                nc.scalar.activation(sg[:, js], psum_t[c][:], ACT.Sigmoid)
                tl, rnd, gt, flr = tl_t[c], rnd_t[c], gt_t[c], flr_t[c]
                nc.vector.tensor_scalar_mul(tl[:], sg[:, js], PAD)
                nc.vector.tensor_scalar(rnd[:], tl[:], MAGIC, MAGIC,
                                        op0=ALU.add, op1=ALU.subtract)
                nc.vector.tensor_tensor(gt[:], rnd[:], tl[:], op=ALU.is_gt)
                nc.vector.tensor_sub(flr[:], rnd[:], gt[:])
                nc.vector.tensor_sub(frh[:, js], tl[:], flr[:])  # fp16 out
                # idx = r0*2048 + c0 (+MAGIC bias, masked off via bitcast)
                t1, ixf = t1_t[c], ixf_t[c]
                r0v = flr[:].rearrange("p (t two) -> p two t", two=2)[:, 0, :]
                c0v = flr[:].rearrange("p (t two) -> p two t", two=2)[:, 1, :]
                nc.vector.tensor_scalar(t1[:], r0v, 2048.0, MAGIC,
                                        op0=ALU.mult, op1=ALU.add)
                nc.vector.tensor_add(ixf[:], t1[:], c0v)
                nc.vector.tensor_single_scalar(idxi[:, TPC * c:TPC * (c + 1)],
                                               ixf[:].bitcast(I32),
                                               0x007FFFFF, op=ALU.bitwise_and)
                for t in range(TPC * c, TPC * (c + 1)):
                    nc.gpsimd.indirect_dma_start(
                        out=S[:, t * SEGP: t * SEGP + SEG],
                        out_offset=None,
                        in_=svb,
                        in_offset=IndirectOffsetOnAxis(ap=idxi[:, t:t + 1], axis=0),
                    )

            # ---- per-half-chunk: bilinear combine + dots -------------------
            Sv = S[:].rearrange("p (t x) -> p t x", t=TPP)

            for h in range(NH):
                THC = HSIZES[h]
                ts = slice(HSTART[h], HSTART[h] + THC)

                def seg_view(off):
                    # [p, t, dj(5 cols, stride 15), 12 = (di,ch)]
                    return Sv[:, ts, off:off + 75].rearrange(
                        "p t (d e) -> p t d e", d=5)[:, :, :, 0:12]

                frv = frh[:, 2 * HSTART[h]:2 * (HSTART[h] + THC)].rearrange(
                    "p (t two) -> p two t", two=2)
                frb = frv[:, 0, :].unsqueeze(2).unsqueeze(3).to_broadcast(
                    (P, THC, 5, 12))
                fcb = frv[:, 1, :].unsqueeze(2).unsqueeze(3).to_broadcast(
                    (P, THC, 4, 12))

                D1v = D1_t[h][:].rearrange("p (t d e) -> p t d e", t=THC, d=5)
                nc.vector.tensor_sub(D1v, seg_view(3), seg_view(0))
                M1v = M1_t[h][:].rearrange("p (t d e) -> p t d e", t=THC, d=5)
                nc.vector.tensor_mul(M1v, D1v, frb)
                Tv = T_t[h][:].rearrange("p (t d e) -> p t d e", t=THC, d=5)
                nc.vector.tensor_add(Tv, M1v, seg_view(0))

                Tf = T_t[h][:].rearrange("p (t x) -> p t x", t=THC)
                T0 = Tf[:, :, 0:48].rearrange("p t (d e) -> p t d e", d=4)
                T12 = Tf[:, :, 12:60].rearrange("p t (d e) -> p t d e", d=4)
                D2v = D2_t[h][:].rearrange("p (t d e) -> p t d e", t=THC, d=4)
                nc.vector.tensor_sub(D2v, T12, T0)
                M2v = M2_t[h][:].rearrange("p (t d e) -> p t d e", t=THC, d=4)
                nc.vector.tensor_mul(M2v, D2v, fcb)
                Uv = U_t[h][:].rearrange("p (t d e) -> p t d e", t=THC, d=4)
                nc.vector.tensor_add(Uv, M2v, T0)

                w2v = W2sb[:].rearrange("p (cl x) -> p cl x", cl=NCLS)[
                    :, :, 48 * HSTART[h]: 48 * (HSTART[h] + THC)]
                mv = dummy[:, 0:NCLS * THC * 48].rearrange(
                    "p (cl x) -> p cl x", cl=NCLS)
                nc.vector.tensor_mul(
                    mv, w2v,
                    U_t[h][:].unsqueeze(1).to_broadcast((P, NCLS, 48 * THC)))
                r2v = r2_t[h][:].rearrange("p (cl o) -> p cl o", cl=NCLS)
                nc.vector.reduce_sum(r2v, mv, axis=AX.X)
                if h == 0:
                    nc.vector.tensor_copy(acc_t[h][:], r2_t[h][:])
                else:
                    nc.vector.tensor_add(acc_t[h][:], acc_t[h - 1][:], r2_t[h][:])

            # ---- final: out[cls] = sum_p accT[p, last, cls] ----------------
            psum2 = psum_t[0][0:1, 0:NCLS]
            nc.tensor.matmul(out=psum2, lhsT=ones[:],
                             rhs=acc_t[NH - 1][:],
                             start=True, stop=True)
            ofin = pool.tile([1, NCLS], F32)
            nc.vector.tensor_add(ofin[:], psum2, b2sb[:])
            nc.sync.dma_start(out, ofin[:])

    nc.compile()
    return nc


@functools.lru_cache(maxsize=2)
def _compiled(num_devices: int, svh: int, svw: int):
    return build_program(num_devices, svh, svw)


def band_layout(img: np.ndarray) -> np.ndarray:
    """[2048, 2048, 3] f32 -> [(2044*2048), 15] fp16 5-row bands."""
    sw = np.lib.stride_tricks.sliding_window_view(img, 5, axis=0)  # [2044,2048,3,5]
    return np.ascontiguousarray(
        sw.transpose(0, 1, 3, 2).astype(np.float16)).reshape(-1, UNIT)


def permute_w2(W2: np.ndarray) -> np.ndarray:
    """Per-patch (i, j, c) -> (j, i, c) to match the kernel's U layout."""
    return np.ascontiguousarray(
        W2.reshape(NCLS, N, PS, PS, 3).transpose(0, 1, 3, 2, 4)
    ).reshape(NCLS, -1).astype(np.float16)


def pack_w1(W1: np.ndarray, b1: np.ndarray) -> np.ndarray:
    """[8192, 48] + [8192] -> [128, 32*128] fp32 PE lhsT tiles.

    k-rows 0:48 = W1 row (p*64+2q); row 48 = b1 even; rows 49:64 zero;
    rows 64:112 = W1 row (p*64+2q+1); row 112 = b1 odd; rest zero."""
    w = W1.reshape(P, 64, 48)       # [p, j, k]
    b = b1.reshape(P, 64)
    o = np.zeros((P, TPP, P), np.float32)
    for q in range(TPP):
        o[0:48, q, :] = w[:, 2 * q, :].T       # [k, p]
        o[48, q, :] = b[:, 2 * q]
        o[64:112, q, :] = w[:, 2 * q + 1, :].T
        o[112, q, :] = b[:, 2 * q + 1]
    return np.ascontiguousarray(o.reshape(P, TPP * P)).astype(np.float32)


def make_in_maps(topview, search_views, W1, b1, W2, b2):
    W1pe = pack_w1(np.asarray(W1, np.float32), np.asarray(b1, np.float32))
    W2p = permute_w2(np.asarray(W2, np.float32))
    b2 = np.ascontiguousarray(b2, np.float32)
    return [{
        "tv": np.ascontiguousarray(topview[i], np.float32),
        "svb": band_layout(np.asarray(search_views[i], np.float32)),
        "W1pe": W1pe, "W2p": W2p, "b2": b2,
    } for i in range(topview.shape[0])]


def kernel(topview, search_views, W1, b1, W2, b2, svh, svw):
    svh, svw = int(svh), int(svw)
    nc = _compiled(B, svh, svw)
    in_maps = make_in_maps(topview, search_views, W1, b1, W2, b2)
    res = bass_utils.run_bass_kernel_spmd(nc, in_maps, core_ids=list(range(B)))
    return np.concatenate([res.results[i]["out"] for i in range(B)], axis=0)


# revision 6
# speedup vs baseline: 1.6453x; 1.0041x over previous
"""Trainium2 Bass kernel for nn_DZSpecimenClfToy — v2.

Per batch item b (B=8, one NeuronCore each):
  1. tv = bilinear_resize(topview[b], (3,64,64)->(3,4,4)) = 0.25*(2x2 sums of
     rows/cols {7,8} of each 16-group); flat[48] in (c,h,w) order.
  2. pre[g] = W1[g,:]@flat + b1[g] (g=8192) on the PE: 32 matmuls of
     lhsT[98,128] x rhs[98,2] (k=0..47 -> W1 row (p*64+2q), k=48..95 ->
     row (p*64+2q+1), k=96/97 -> b1 via eye2 columns in rhs).
  3. coords = sigmoid(pre) (ACT, from PSUM); tl = coords*2043; floor/frac via
     the 2^23 magic trick; gather index = r0*2048 + c0.
  4. Search view pre-laid out on host as 5-row bands (fp16):
     svb16[r0*2048 + c0] = img[r0:r0+5, c0, :] -> one patch support =
     5 consecutive 15-elem column blocks = 75 contiguous fp16.
     32 indirect DMAs (one address per partition each) gather all 4096.
  5. Bilinear combine in fp16 on DVE (row lerp then col lerp).
  6. Per-class dot via tensor_tensor_reduce with cross-chunk accumulator
     chaining (init = b2/128); final partition sum via ones-matmul on PE.

Sharding: data-parallel over batch across 8 cores; weights replicated.
"""
import functools
from contextlib import ExitStack

import numpy as np

import concourse.bass as bass
import concourse.tile as tile
from concourse import bacc, mybir
import concourse.bass_utils as bass_utils
from concourse.bass import IndirectOffsetOnAxis

F32 = mybir.dt.float32
F16 = mybir.dt.float16
I32 = mybir.dt.int32
ALU = mybir.AluOpType
ACT = mybir.ActivationFunctionType
AX = mybir.AxisListType

B = 8          # batch == number of cores
H = W = 2048   # search view height/width
N = 4096       # patches per item
PS = 4         # patch size
NCLS = 2       # classes
P = 128        # partitions
TPP = N // P   # patches per partition = 32
NCH = 8        # pipeline chunks
TPC = TPP // NCH  # patches per partition per chunk = 4

NB = H - PS    # 2044 bands of 5 rows (r0 in 0..2043; used r0 <= 2042)
UNIT = 15      # fp16 elems per (band,col) unit: 5 rows x 3 ch
SEG = 75       # gathered fp16 per patch (5 column units)
SEGP = 80      # SBUF stride per patch segment
MAGIC = 8388608.0  # 2**23
PAD = 2043.0   # (svh-1) - ps


def build_program(num_devices: int, svh: int, svw: int):
    assert svh == H and svw == W, (svh, svw)

    nc = bacc.Bacc("TRN2", target_bir_lowering=False, debug=False,
                   enable_asserts=False, num_devices=num_devices)

    tv = nc.dram_tensor("tv", [3, 64, 64], F32, kind="ExternalInput").ap()
    svb = nc.dram_tensor("svb", [NB * W, UNIT], F16, kind="ExternalInput").ap()
    w1pe = nc.dram_tensor("W1pe", [P, TPP * P], F32, kind="ExternalInput").ap()
    w2 = nc.dram_tensor("W2p", [NCLS, N * PS * PS * 3], F16, kind="ExternalInput").ap()
    b2 = nc.dram_tensor("b2", [NCLS], F32, kind="ExternalInput").ap()
    out = nc.dram_tensor("out", [1, NCLS], F32, kind="ExternalOutput").ap()

    with tile.TileContext(nc) as tc:
        with ExitStack() as ctx:
            pool = ctx.enter_context(tc.tile_pool(name="main", bufs=1))
            ppool = ctx.enter_context(tc.tile_pool(name="ps", bufs=1, space="PSUM"))

            # ---- input DMAs ------------------------------------------------
            # Topview rows {7,8},{23,24},{39,40},{55,56}: 128 contiguous floats
            # per (c, rowgroup) starting at row 7 of each 16-row group.
            A = pool.tile([1, 1536], F32)
            tv_sel = tv.rearrange("c (k s) w -> c k (s w)", s=16)[:, :, 7 * 64:9 * 64]
            nc.sync.dma_start(A[:].rearrange("p (c k x) -> p c k x", c=3, k=4),
                              tv_sel.unsqueeze(0))

            W1sb = pool.tile([P, TPP * P], F32)
            nc.sync.dma_start(W1sb[:, 0:2048], w1pe[:, 0:2048])
            nc.sync.dma_start(W1sb[:, 2048:4096], w1pe[:, 2048:4096])

            W2sb = pool.tile([P, NCLS * 1536], F16)  # [p, c*1536 + t*48 + x]
            nc.sync.dma_start(W2sb[:].rearrange("p (c x) -> p c x", c=NCLS),
                              w2.rearrange("c (p x) -> p c x", p=P))

            b2sb = pool.tile([1, NCLS], F32)
            nc.sync.dma_start(b2sb[:], b2.unsqueeze(0))

            ones = pool.tile([P, 1], F32)
            nc.vector.memset(ones[:], 1.0)

            # ---- topview 64x64 -> 4x4 resize -> flat16 ---------------------
            V = pool.tile([1, 768], F32)           # row-pair sums
            A4 = A[:].rearrange("p (ck r w) -> p ck r w", ck=12, r=2)
            nc.vector.tensor_add(V[:].rearrange("p (ck w) -> p ck w", ck=12),
                                 A4[:, :, 0, :], A4[:, :, 1, :])
            F48 = pool.tile([1, 48], F32)
            V4 = V[:].rearrange("p (ck g s) -> p ck g s", ck=12, g=4)
            nc.vector.tensor_add(F48[:].rearrange("p (ck g) -> p ck g", ck=12),
                                 V4[:, :, :, 7], V4[:, :, :, 8])
            # flat2x = [flat*0.25, 1, pad0..; flat*0.25, 1, pad0..] on one
            # partition; rows 49:64 / 113:128 multiply zero lhsT rows.
            flat2x = pool.tile([1, P], F32)
            nc.vector.memset(flat2x[:], 0.0)
            nc.vector.tensor_scalar_mul(flat2x[:, 0:48], F48[:], 0.25)
            nc.vector.memset(flat2x[:, 48:49], 1.0)
            nc.vector.tensor_scalar_mul(flat2x[:, 64:112], F48[:], 0.25)
            nc.vector.memset(flat2x[:, 112:113], 1.0)

            # ---- rhs [128, 2] built on-chip: K=1 matmul transposes flat2x
            # onto partitions, then two 64-aligned DVE column copies.
            # (PSUM banks are scarce: borrow chunk 7's matvec bank — its
            # matmuls overwrite it only after the rhs copies have read it.)
            CSIZES = [1, 1, 2, 4, 6, 6, 6, 6]     # matmuls per chunk
            CSTART = [sum(CSIZES[:i]) for i in range(NCH)]
            psum_t = [ppool.tile([P, 2 * CSIZES[c]], F32, name=f"psum{c}")
                      for c in range(NCH)]
            psumT = psum_t[NCH - 1][:, 0:1]
            nc.tensor.matmul(out=psumT, lhsT=flat2x[:], rhs=ones[0:1, 0:1],
                             start=True, stop=True)
            # warm-up matmul: keeps the PE pipeline hot while the rhs copies
            # land, so the first matvec matmul issues without the wake gap.
            nc.tensor.matmul(out=psum_t[NCH - 1][0:1, 2:4],
                             lhsT=flat2x[0:1, 0:1], rhs=flat2x[0:1, 0:2],
                             start=True, stop=True)
            rhs = pool.tile([P, 2], F32)
            nc.vector.memset(rhs[:], 0.0)
            nc.vector.tensor_copy(rhs[0:64, 0:1], psumT[0:64, 0:1])
            nc.vector.tensor_copy(rhs[64:128, 1:2], psumT[64:128, 0:1])

            # ---- PE matvec: psum_c[p, j] = pre[p*64+j] + b1 (per chunk) ----
            for q in range(TPP):
                c = max(i for i in range(NCH) if CSTART[i] <= q)
                qq = q - CSTART[c]
                nc.tensor.matmul(out=psum_t[c][:, 2 * qq:2 * qq + 2],
                                 lhsT=W1sb[:, P * q:P * (q + 1)],
                                 rhs=rhs[:], start=True, stop=True)

            sg = pool.tile([P, 64], F32)
            frh = pool.tile([P, 64], F16)   # fp16 fractions, (t, two) interleaved
            idxi = pool.tile([P, TPP], I32)
            S = pool.tile([P, TPP * SEGP], F16)

            # scratch per chunk
            def cpool(name, w, dt=F32):
                return [pool.tile([P, w], dt, name=f"{name}{c}")
                        for c in range(NCH)]

            tl_t = [pool.tile([P, 2 * CSIZES[c]], F32, name=f"tl{c}")
                    for c in range(NCH)]
            rnd_t = [pool.tile([P, 2 * CSIZES[c]], F32, name=f"rnd{c}")
                     for c in range(NCH)]
            gt_t = [pool.tile([P, 2 * CSIZES[c]], F32, name=f"gt{c}")
                    for c in range(NCH)]
            flr_t = [pool.tile([P, 2 * CSIZES[c]], F32, name=f"flr{c}")
                     for c in range(NCH)]
            t1_t = [pool.tile([P, CSIZES[c]], F32, name=f"t1{c}")
                    for c in range(NCH)]
            ixf_t = [pool.tile([P, CSIZES[c]], F32, name=f"ixf{c}")
                     for c in range(NCH)]

            # combine splits: six halves of 4 patches + four of 2 (smaller
            # tail chunks shrink the exposed post-train combine time)
            HSIZES = [4, 4, 4, 4, 4, 4, 2, 2, 2, 1, 1]
            HSTART = [sum(HSIZES[:i]) for i in range(len(HSIZES))]
            NH = len(HSIZES)

            def hpool(name, scale, dt=F16):
                return [pool.tile([P, HSIZES[h] * scale], dt, name=f"{name}{h}")
                        for h in range(NH)]

            D1_t = hpool("D1", 60)
            M1_t = hpool("M1", 60)
            T_t = hpool("T", 60)
            D2_t = hpool("D2", 48)
            M2_t = hpool("M2", 48)
            U_t = hpool("U", 48)
            dummy = pool.tile([P, NCLS * 4 * 48], F16)
            r2_t = [pool.tile([P, NCLS], F32, name=f"r2{h}") for h in range(NH)]
            acc_t = [pool.tile([P, NCLS], F32, name=f"acc{h}") for h in range(NH)]

            # ---- per-chunk: sigmoid -> coords -> gather indices ------------
            for c in range(NCH):
                js = slice(2 * CSTART[c], 2 * (CSTART[c] + CSIZES[c]))
                nc.scalar.activation(sg[:, js], psum_t[c][:], ACT.Sigmoid)
                tl, rnd, gt, flr = tl_t[c], rnd_t[c], gt_t[c], flr_t[c]
                nc.vector.tensor_scalar_mul(tl[:], sg[:, js], PAD)
                nc.vector.tensor_scalar(rnd[:], tl[:], MAGIC, MAGIC,
                                        op0=ALU.add, op1=ALU.subtract)
                nc.vector.tensor_tensor(gt[:], rnd[:], tl[:], op=ALU.is_gt)
                nc.vector.tensor_sub(flr[:], rnd[:], gt[:])
                nc.vector.tensor_sub(frh[:, js], tl[:], flr[:])  # fp16 out
                # idx = r0*2048 + c0 (+MAGIC bias, masked off via bitcast)
                t1, ixf = t1_t[c], ixf_t[c]
                r0v = flr[:].rearrange("p (t two) -> p two t", two=2)[:, 0, :]
                c0v = flr[:].rearrange("p (t two) -> p two t", two=2)[:, 1, :]
                nc.vector.tensor_scalar(t1[:], r0v, 2048.0, MAGIC,
                                        op0=ALU.mult, op1=ALU.add)
                nc.vector.tensor_add(ixf[:], t1[:], c0v)
                nc.vector.tensor_single_scalar(
                    idxi[:, CSTART[c]:CSTART[c] + CSIZES[c]],
                    ixf[:].bitcast(I32), 0x007FFFFF, op=ALU.bitwise_and)
                for t in range(CSTART[c], CSTART[c] + CSIZES[c]):
                    nc.gpsimd.indirect_dma_start(
                        out=S[:, t * SEGP: t * SEGP + SEG],
                        out_offset=None,
                        in_=svb,
                        in_offset=IndirectOffsetOnAxis(ap=idxi[:, t:t + 1], axis=0),
                    )

            # ---- per-half-chunk: bilinear combine + dots -------------------
            Sv = S[:].rearrange("p (t x) -> p t x", t=TPP)

            for h in range(NH):
                THC = HSIZES[h]
                ts = slice(HSTART[h], HSTART[h] + THC)

                def seg_view(off):
                    # [p, t, dj(5 cols, stride 15), 12 = (di,ch)]
                    return Sv[:, ts, off:off + 75].rearrange(
                        "p t (d e) -> p t d e", d=5)[:, :, :, 0:12]

                frv = frh[:, 2 * HSTART[h]:2 * (HSTART[h] + THC)].rearrange(
                    "p (t two) -> p two t", two=2)
                frb = frv[:, 0, :].unsqueeze(2).unsqueeze(3).to_broadcast(
                    (P, THC, 5, 12))
                fcb = frv[:, 1, :].unsqueeze(2).unsqueeze(3).to_broadcast(
                    (P, THC, 4, 12))

                D1v = D1_t[h][:].rearrange("p (t d e) -> p t d e", t=THC, d=5)
                nc.vector.tensor_sub(D1v, seg_view(3), seg_view(0))
                M1v = M1_t[h][:].rearrange("p (t d e) -> p t d e", t=THC, d=5)
                nc.vector.tensor_mul(M1v, D1v, frb)
                Tv = T_t[h][:].rearrange("p (t d e) -> p t d e", t=THC, d=5)
                nc.vector.tensor_add(Tv, M1v, seg_view(0))

                Tf = T_t[h][:].rearrange("p (t x) -> p t x", t=THC)
                T0 = Tf[:, :, 0:48].rearrange("p t (d e) -> p t d e", d=4)
                T12 = Tf[:, :, 12:60].rearrange("p t (d e) -> p t d e", d=4)
                D2v = D2_t[h][:].rearrange("p (t d e) -> p t d e", t=THC, d=4)
                nc.vector.tensor_sub(D2v, T12, T0)
                M2v = M2_t[h][:].rearrange("p (t d e) -> p t d e", t=THC, d=4)
                nc.vector.tensor_mul(M2v, D2v, fcb)
                Uv = U_t[h][:].rearrange("p (t d e) -> p t d e", t=THC, d=4)
                nc.vector.tensor_add(Uv, M2v, T0)

                w2v = W2sb[:].rearrange("p (cl x) -> p cl x", cl=NCLS)[
                    :, :, 48 * HSTART[h]: 48 * (HSTART[h] + THC)]
                mv = dummy[:, 0:NCLS * THC * 48].rearrange(
                    "p (cl x) -> p cl x", cl=NCLS)
                nc.vector.tensor_mul(
                    mv, w2v,
                    U_t[h][:].unsqueeze(1).to_broadcast((P, NCLS, 48 * THC)))
                r2v = r2_t[h][:].rearrange("p (cl o) -> p cl o", cl=NCLS)
                nc.vector.reduce_sum(r2v, mv, axis=AX.X)
                if h == 0:
                    nc.vector.tensor_copy(acc_t[h][:], r2_t[h][:])
                else:
                    nc.vector.tensor_add(acc_t[h][:], acc_t[h - 1][:], r2_t[h][:])

            # ---- final: out[cls] = sum_p accT[p, last, cls] ----------------
            psum2 = psum_t[0][0:1, 0:NCLS]
            nc.tensor.matmul(out=psum2, lhsT=ones[:],
                             rhs=acc_t[NH - 1][:],
                             start=True, stop=True)
            ofin = pool.tile([1, NCLS], F32)
            nc.vector.tensor_add(ofin[:], psum2, b2sb[:])
            nc.sync.dma_start(out, ofin[:])

    nc.compile()
    return nc


@functools.lru_cache(maxsize=2)
def _compiled(num_devices: int, svh: int, svw: int):
    return build_program(num_devices, svh, svw)


def band_layout(img: np.ndarray) -> np.ndarray:
    """[2048, 2048, 3] f32 -> [(2044*2048), 15] fp16 5-row bands."""
    sw = np.lib.stride_tricks.sliding_window_view(img, 5, axis=0)  # [2044,2048,3,5]
    return np.ascontiguousarray(
        sw.transpose(0, 1, 3, 2).astype(np.float16)).reshape(-1, UNIT)


def permute_w2(W2: np.ndarray) -> np.ndarray:
    """Per-patch (i, j, c) -> (j, i, c) to match the kernel's U layout."""
    return np.ascontiguousarray(
        W2.reshape(NCLS, N, PS, PS, 3).transpose(0, 1, 3, 2, 4)
    ).reshape(NCLS, -1).astype(np.float16)


def pack_w1(W1: np.ndarray, b1: np.ndarray) -> np.ndarray:
    """[8192, 48] + [8192] -> [128, 32*128] fp32 PE lhsT tiles.

    k-rows 0:48 = W1 row (p*64+2q); row 48 = b1 even; rows 49:64 zero;
    rows 64:112 = W1 row (p*64+2q+1); row 112 = b1 odd; rest zero."""
    w = W1.reshape(P, 64, 48)       # [p, j, k]
    b = b1.reshape(P, 64)
    o = np.zeros((P, TPP, P), np.float32)
    for q in range(TPP):
        o[0:48, q, :] = w[:, 2 * q, :].T       # [k, p]
        o[48, q, :] = b[:, 2 * q]
        o[64:112, q, :] = w[:, 2 * q + 1, :].T
        o[112, q, :] = b[:, 2 * q + 1]
    return np.ascontiguousarray(o.reshape(P, TPP * P)).astype(np.float32)


def make_in_maps(topview, search_views, W1, b1, W2, b2):
    W1pe = pack_w1(np.asarray(W1, np.float32), np.asarray(b1, np.float32))
    W2p = permute_w2(np.asarray(W2, np.float32))
    b2 = np.ascontiguousarray(b2, np.float32)
    return [{
        "tv": np.ascontiguousarray(topview[i], np.float32),
        "svb": band_layout(np.asarray(search_views[i], np.float32)),
        "W1pe": W1pe, "W2p": W2p, "b2": b2,
    } for i in range(topview.shape[0])]


def kernel(topview, search_views, W1, b1, W2, b2, svh, svw):
    svh, svw = int(svh), int(svw)
    nc = _compiled(B, svh, svw)
    in_maps = make_in_maps(topview, search_views, W1, b1, W2, b2)
    res = bass_utils.run_bass_kernel_spmd(nc, in_maps, core_ids=list(range(B)))
    return np.concatenate([res.results[i]["out"] for i in range(B)], axis=0)
